# revision 8
# baseline (speedup 1.0000x reference)
"""Trainium2 Bass kernel for nn_AttentionLayer (GNN attention-coefficient layer).

Math (reference):
    s = BN_train(self @ W + b);  n = BN_train(neigh @ W + b)   (stats over batch)
    logits = relu(concat([s_bcast, n]) @ W_out + b_out)
    coeff  = softmax_k(logits)                                  -> [N, K, 1]

Folded form used here: with u = W_out[:A,0], v = W_out[A:,0],
    logit[i,k] = relu( ys[i]@ws + yn[i,k]@wn + C )
where ys = xs@W, yn = xn@W (b_shared cancels in training-mode BN),
ws = inv_s*gamma*u, wn = inv_n*gamma*v, inv = rsqrt(var+eps), and
C = sum_a[(beta - mean_s*inv_s*gamma)*u + (beta - mean_n*inv_n*gamma)*v] + b_out.

Device plan (SPMD over 8 cores, nodes sharded):
  phase 1: stream Xt (host-pretransposed, neigh in k-major [F,K,nodes] order,
           fp8e3 scaled x2 for halved HBM traffic; self stays fp16) ->
           yT = W^T Xt in PSUM (512-col subtiles, 2 accumulating matmuls each);
           ACT copies yT (with the 1/2 dequant folded into the copy scale) to a
           persistent fp16 SBUF store with sum(y) accum; DVE squares via
           tensor_tensor (2x mode) + tensor_scalar accumulate (4x mode).
  stats:   per-feature sums -> local mean/E[y^2] prescaled by 1/(8*rows);
           tiny AllReduce(add) across the 8 cores; wn/ws/C on-chip
           (rsqrt via exp(-0.5 ln); C reduced+broadcast by one ones-matmul).
  phase 2: per 128-node block: 32 matmuls (lhsT = CONTIGUOUS 128-col slices of
           the k-major yT store -> FWL fast weight load, rhs = wn) -> t PSUM
           [nodes, 32]; ACT relu(t + a_bias); exp; DVE row-sum + reciprocal;
           ACT scale; DMA out.
"""

import numpy as np
import ml_dtypes

import concourse.bass as bass
import concourse.mybir as mybir
import concourse.tile as tile
from concourse import bacc
from concourse.bass_utils import run_bass_kernel_spmd

N_CORES = 8
N_FULL, K, F, A = 20000, 32, 256, 128
BN_EPS = 1e-3
XSCALE = 2.0  # host premultiplies neigh feats before the e3m4 cast

F8 = mybir.dt.float8e3
F16 = mybir.dt.float16
F32 = mybir.dt.float32
AF = mybir.ActivationFunctionType

# Knobs for the test harness.
PROFILE = False
LAST_RESULT = None


def build_nc(nodes, k=K, f=F, a=A, n_cores=N_CORES, row_tile=4096, sub=512):
    """Build the per-core SPMD program. `nodes` = nodes per core."""
    assert f == 2 * 128 and a == 128
    rows_n = nodes * k
    rows_s = nodes
    nblk = (nodes + 127) // 128

    def tiles_of(rows):
        out = []
        r = 0
        while r < rows:
            out.append((r, min(row_tile, rows - r)))
            r += row_tile
        return out

    grp = 4 * sub  # 2048-col psum groups (4 banks)

    def pairs_of(rows):
        # (base, ns) for each up-to-2048-col psum group
        out = []
        for r0, nr in tiles_of(rows):
            s0 = 0
            while s0 < nr:
                ns = min(grp, nr - s0)
                out.append((r0 + s0, ns))
                s0 += ns
        return out

    np_n = len(pairs_of(rows_n))
    np_s = len(pairs_of(rows_s))

    nc = bacc.Bacc("TRN2", target_bir_lowering=False, num_devices=n_cores)
    xt_n = nc.declare_dram_parameter("xt_n", [f, rows_n], mybir.dt.uint8, isOutput=False)
    xt_s = nc.declare_dram_parameter("xt_s", [f, rows_s], F16, isOutput=False)
    w_lhsT = nc.declare_dram_parameter("w_lhsT", [2, 128, a], F16, isOutput=False)
    # params columns: gamma, v, u, b_out/A, beta*v, beta*u
    params = nc.declare_dram_parameter("params", [a, 6], F32, isOutput=False)
    out_d = nc.declare_dram_parameter("out", [rows_s, k], F32, isOutput=True)

    from contextlib import ExitStack

    with tile.TileContext(nc) as tc, ExitStack() as ctx:
        singles = ctx.enter_context(tc.tile_pool(name="singles", bufs=1))
        xn_pool = ctx.enter_context(tc.tile_pool(name="xn_pool", bufs=2))
        sq_pool = ctx.enter_context(tc.tile_pool(name="sq_pool", bufs=2))
        p2_pool = ctx.enter_context(tc.tile_pool(name="p2_pool", bufs=3))
        psum = ctx.enter_context(tc.tile_pool(name="psum", bufs=2, space="PSUM"))
        dram = ctx.enter_context(tc.tile_pool(name="dram", bufs=1, space="DRAM"))

        # ---- setup: params and weights
        w_sb = singles.tile([128, 2, a], F16)
        nc.sync.dma_start(out=w_sb, in_=w_lhsT.ap().rearrange("c p a -> p c a"))
        params_sb = singles.tile([a, 6], F32)
        nc.sync.dma_start(out=params_sb, in_=params.ap())
        eps_sb = singles.tile([a, 1], F32)
        nc.vector.memset(eps_sb, BN_EPS)
        ones_sb = singles.tile([a, 1], F32)
        nc.vector.memset(ones_sb, 1.0)
        ones_128 = singles.tile([a, 128], F16)
        nc.vector.memset(ones_128, 1.0)
        # warm the ACT function tables off the critical path; end with Ln so
        # its table set is resident when the post-collective chain issues Ln
        # (phase-1 Copy ops don't swap table sets)
        warm_sb = singles.tile([a, 1], F32)
        nc.scalar.activation(out=warm_sb, in_=ones_sb, func=AF.Exp)
        nc.scalar.activation(out=warm_sb, in_=ones_sb, func=AF.Relu)
        nc.scalar.activation(out=warm_sb, in_=ones_sb, func=AF.Ln)

        cv3 = singles.tile([a, 3], F32)
        nc.vector.tensor_copy(out=cv3[:, 2:3], in_=params_sb[:, 3:4])

        # ---- persistent stores (padded so 128-col lhsT slices never run out)
        yt_store = singles.tile([a, rows_n + 128], F16)
        ys_store = singles.tile([a, rows_s + 128], F16)

        # per-pair sum(y) columns (from the ACT copy's accum_out, already in
        # dequantized units and prescaled at reduce time) and per-pair
        # sum(y^2) columns (from the DVE square+accumulate pair)
        sum_n = singles.tile([a, np_n], F32)
        sum_s = singles.tile([a, np_s], F32)
        sq_n = singles.tile([a, np_n], F32)
        sq_s = singles.tile([a, np_s], F32)
        red_junk = singles.tile([a, max(np_n, np_s)], F32)

        # ---- phase 1: stream X^T, matmul into yT (paired 512-col subtiles per
        # 2-bank PSUM tile), one ACT copy (dequant scale + sum accum) per pair,
        # one DVE square-with-accum (scalar_tensor_tensor) per group
        def stream(xt_dram, dt, rows, store, sums, sqs, drain_scale, pool):
            view = xt_dram.ap().rearrange("(c p) r -> p c r", p=128)
            ipair = 0
            for r0, nr in tiles_of(rows):
                xt_t = pool.tile([128, 2, row_tile], dt, tag="xt")
                dst = xt_t[:, :, :nr]
                if dt == F8:
                    dst = dst.bitcast(mybir.dt.uint8)
                nc.sync.dma_start(out=dst, in_=view[:, :, r0 : r0 + nr])
                s0 = 0
                while s0 < nr:
                    ns = min(grp, nr - s0)
                    yt_psum = psum.tile([a, grp], F32, tag="g")
                    for q0 in range(0, ns, sub):
                        qn = min(sub, ns - q0)
                        for c in range(2):
                            nc.tensor.matmul(
                                yt_psum[:, q0 : q0 + qn],
                                w_sb[:, c, :],
                                xt_t[:, c, s0 + q0 : s0 + q0 + qn],
                                start=(c == 0),
                                stop=(c == 1),
                            )
                    base = r0 + s0
                    nc.scalar.activation(
                        out=store[:, base : base + ns],
                        in_=yt_psum[:, :ns],
                        func=AF.Copy,
                        scale=drain_scale,
                        accum_out=sums[:, ipair : ipair + 1],
                    )
                    src = store[:, base : base + ns]
                    scr = sq_pool.tile([a, grp], F16, tag="sq")
                    if ipair % 6 == 5:
                        # shift ~1/6 of the square work to ACT to balance engines
                        nc.scalar.activation(
                            out=scr[:, :ns],
                            in_=src,
                            func=AF.Square,
                            accum_out=sqs[:, ipair : ipair + 1],
                        )
                    else:
                        nc.vector.scalar_tensor_tensor(
                            out=scr[:, :ns],
                            in0=src,
                            scalar=1.0,
                            in1=src,
                            op0=mybir.AluOpType.mult,
                            op1=mybir.AluOpType.mult,
                            accum_out=sqs[:, ipair : ipair + 1],
                        )
                    ipair += 1
                    s0 += ns
            return ipair

        # allred_in layout: [mean_n, mean_s, E2_n, E2_s], prescaled by
        # 1/(n_cores*rows) so the AllReduce(add) yields global stats directly
        allred_in = singles.tile([a, 4], F32)

        def finish_stats(sums, sqs, n_pairs_used, rows, col):
            inv = 1.0 / (rows * n_cores)
            nc.vector.tensor_scalar(
                out=red_junk[:, :n_pairs_used],
                in0=sums[:, :n_pairs_used],
                scalar1=inv,
                scalar2=0.0,
                op0=mybir.AluOpType.mult,
                op1=mybir.AluOpType.add,
                accum_out=allred_in[:, col : col + 1],
            )
            nc.vector.tensor_scalar(
                out=red_junk[:, :n_pairs_used],
                in0=sqs[:, :n_pairs_used],
                scalar1=inv,
                scalar2=0.0,
                op0=mybir.AluOpType.mult,
                op1=mybir.AluOpType.add,
                accum_out=allred_in[:, col + 2 : col + 3],
            )

        # self stream first: its stats ops clear the DVE queue while the long
        # neigh stream runs, so the pre-collective DVE tail is minimal
        used_s = stream(xt_s, F16, rows_s, ys_store, sum_s, sq_s, 1.0, singles)
        finish_stats(sum_s, sq_s, used_s, rows_s, 1)
        used_n = stream(xt_n, F8, rows_n, yt_store, sum_n, sq_n, 1.0 / XSCALE, xn_pool)
        finish_stats(sum_n, sq_n, used_n, rows_n, 0)

        cc_in = dram.tile([a, 4], F32)
        cc_out = dram.tile([a, 4], F32)
        nc.sync.dma_start(out=cc_in, in_=allred_in)
        nc.gpsimd.collective_compute(
            "AllReduce",
            mybir.AluOpType.add,
            replica_groups=[list(range(n_cores))],
            ins=[cc_in.opt()],
            outs=[cc_out.opt()],
        )
        gs = singles.tile([a, 4], F32)
        nc.sync.dma_start(out=gs, in_=cc_out)

        # ---- global mean/var -> inv, w-vectors, constant C
        # params_sb columns: 0 gamma, 1 v, 2 u, 3 b_out/128, 4 beta*v, 5 beta*u
        # rsqrt via exp(-0.5*log(var+eps)) to stay in the Exp ACT table set
        # (avoids two Sqrt table-set switches on the critical path).
        gmean = gs[:, 0:2]
        # nvar = mean^2 - E2 (one STT per column); Ln(-1*nvar + eps) = ln(var+eps)
        nvar = singles.tile([a, 2], F32)
        for c in range(2):
            nc.vector.scalar_tensor_tensor(
                out=nvar[:, c : c + 1],
                in0=gmean[:, c : c + 1],
                scalar=gmean[:, c : c + 1],
                in1=gs[:, 2 + c : 3 + c],
                op0=mybir.AluOpType.mult,
                op1=mybir.AluOpType.subtract,
            )
        lv = singles.tile([a, 2], F32)
        nc.scalar.activation(out=lv, in_=nvar, func=AF.Ln, scale=-1.0, bias=eps_sb)
        inv = singles.tile([a, 2], F32)
        nc.scalar.activation(out=inv, in_=lv, func=AF.Exp, scale=-0.5)

        # wf = (inv * gamma) * [v | u] in one STT
        wf = singles.tile([a, 2], F32)  # col0: wn, col1: ws
        nc.vector.scalar_tensor_tensor(
            out=wf,
            in0=inv,
            scalar=params_sb[:, 0:1],
            in1=params_sb[:, 1:3],
            op0=mybir.AluOpType.mult,
            op1=mybir.AluOpType.mult,
        )
        w2_sb = singles.tile([a, 2], F16)
        nc.vector.tensor_copy(out=w2_sb, in_=wf)
        wn_sb = w2_sb[:, 0:1]
        ws_sb = w2_sb[:, 1:2]

        # C vector: beta*[v|u] - mean*wf  (mean*inv*gamma*[v|u] == gmean*wf)
        tmu = singles.tile([a, 2], F32)
        nc.vector.tensor_mul(tmu, gmean, wf)
        nc.vector.tensor_sub(cv3[:, 0:2], params_sb[:, 4:6], tmu)
        cvec = singles.tile([a, 1], F32)
        nc.vector.reduce_sum(out=cvec, in_=cv3, axis=mybir.AxisListType.X)
        cvec16 = singles.tile([a, 1], F16)
        nc.vector.tensor_copy(out=cvec16, in_=cvec)

        # reduce over partitions AND broadcast the scalar in one matmul:
        # out[m, 0] = sum_a ones[a, m] * cvec[a, 0]
        cb_tile = psum.tile([a, grp], F32, tag="g")
        nc.tensor.matmul(cb_tile[:, 0:1], ones_128, cvec16, start=True, stop=True)
        c_bcast = singles.tile([a, 1], F32)
        nc.vector.tensor_copy(out=c_bcast, in_=cb_tile[:, 0:1])

        # ---- phase 2: a_i = ys . ws + C, then t matmuls + softmax, per block
        # (the a-matmul is interleaved into the block loop so it pipelines
        # with the t-matmuls instead of forming a serial prologue).
        # lhsT slices are always full 128 contiguous cols (stores are padded)
        # so the compiler's FWL fast-weight-load stays enabled.
        a_all = singles.tile([128, nblk], F32)
        for b in range(nblk):
            b0 = b * 128
            nb = min(128, nodes - b0)
            blk_psum = psum.tile([128, grp], F32, tag="g")
            t_psum = blk_psum[:, 0:k]
            nc.tensor.matmul(
                blk_psum[:, sub : sub + 1],
                ys_store[:, b0 : b0 + 128],
                ws_sb,
                start=True,
                stop=True,
            )
            nc.vector.tensor_add(a_all[:, b : b + 1], blk_psum[:, sub : sub + 1], c_bcast)
            for kk in range(k):
                nc.tensor.matmul(
                    t_psum[:, kk : kk + 1],
                    yt_store[:, kk * nodes + b0 : kk * nodes + b0 + 128],
                    wn_sb,
                    start=True,
                    stop=True,
                )
            e_raw = p2_pool.tile([128, k], F32, tag="l")
            nc.scalar.activation(
                out=e_raw[:nb, :],
                in_=t_psum[:nb, :],
                func=AF.Exp,
                bias=a_all[:nb, b : b + 1],
            )
            e_sb = p2_pool.tile([128, k], F32, tag="e")
            nc.vector.tensor_scalar_max(e_sb[:nb, :], e_raw[:nb, :], 1.0)
            ssum = p2_pool.tile([128, 1], F32, tag="ssum")
            nc.vector.reduce_sum(out=ssum[:nb, :], in_=e_sb[:nb, :], axis=mybir.AxisListType.X)
            rec = p2_pool.tile([128, 1], F32, tag="rec")
            nc.vector.reciprocal(out=rec[:nb, :], in_=ssum[:nb, :])
            coeff = p2_pool.tile([128, k], F32, tag="coeff")
            nc.scalar.activation(
                out=coeff[:nb, :], in_=e_sb[:nb, :], func=AF.Copy, scale=rec[:nb, :]
            )
            nc.sync.dma_start(out=out_d[b0 : b0 + nb, :], in_=coeff[:nb, :])

    nc.compile()
    return nc


_NC_CACHE = {}


def _get_nc(nodes, row_tile=4096):
    key = (nodes, row_tile)
    if key not in _NC_CACHE:
        _NC_CACHE[key] = build_nc(nodes, row_tile=row_tile)
    return _NC_CACHE[key]


def make_in_maps(self_feats, neigh_feats, W_shared, gamma, beta, W_out, b_out, n_cores=N_CORES):
    n = self_feats.shape[0]
    nodes = n // n_cores
    w_lhsT = np.stack([W_shared[:128], W_shared[128:]]).astype(np.float16)
    gamma = np.asarray(gamma, np.float32)
    beta = np.asarray(beta, np.float32)
    u = np.asarray(W_out[:A, 0], np.float32)
    v = np.asarray(W_out[A:, 0], np.float32)
    # columns: gamma, v, u, b_out/A, beta*v, beta*u
    params = np.stack(
        [
            gamma,
            v,
            u,
            np.full(A, np.float32(np.asarray(b_out).reshape(-1)[0]) / A),
            beta * v,
            beta * u,
        ],
        axis=1,
    ).astype(np.float32)
    in_maps = []
    for c in range(n_cores):
        sl = slice(c * nodes, (c + 1) * nodes)
        xs = np.asarray(self_feats[sl], np.float32)
        # k-major: [F, K, nodes] so phase-2 lhsT slices are contiguous
        xn = np.asarray(neigh_feats[sl], np.float32).transpose(2, 1, 0)
        xn8 = np.ascontiguousarray(xn.reshape(F, nodes * K) * XSCALE).astype(
            ml_dtypes.float8_e3m4
        )
        in_maps.append(
            {
                "xt_n": xn8.view(np.uint8),
                "xt_s": np.ascontiguousarray(xs.T).astype(np.float16),
                "w_lhsT": w_lhsT,
                "params": params,
            }
        )
    return in_maps


def kernel(self_feats, neigh_feats, W_shared, b_shared, gamma, beta, W_out, b_out):
    global LAST_RESULT
    self_feats = np.asarray(self_feats, np.float32)
    neigh_feats = np.asarray(neigh_feats, np.float32)
    W_shared = np.asarray(W_shared, np.float32)
    gamma = np.asarray(gamma, np.float32)
    beta = np.asarray(beta, np.float32)
    W_out = np.asarray(W_out, np.float32)
    b_out = np.asarray(b_out, np.float32)
    n = self_feats.shape[0]
    nodes = n // N_CORES
    nc = _get_nc(nodes)
    in_maps = make_in_maps(self_feats, neigh_feats, W_shared, gamma, beta, W_out, b_out)
    kw = {}
    if PROFILE:
        kw = dict(trace=True, trace_cores=[0])
    res = run_bass_kernel_spmd(nc, in_maps, list(range(N_CORES)), **kw)
    LAST_RESULT = res
    out = np.concatenate([res.results[c]["out"] for c in range(N_CORES)], axis=0)
    return out[:, :, None].astype(np.float32)


# revision 9
# speedup vs baseline: 1.2137x; 1.2137x over previous
"""Trainium2 Bass kernel for nn_AttentionLayer (GNN attention-coefficient layer).

Math (reference):
    s = BN_train(self @ W + b);  n = BN_train(neigh @ W + b)   (stats over batch)
    logits = relu(concat([s_bcast, n]) @ W_out + b_out)
    coeff  = softmax_k(logits)                                  -> [N, K, 1]

Folded form used here: with u = W_out[:A,0], v = W_out[A:,0],
    logit[i,k] = relu( ys[i]@ws + yn[i,k]@wn + C )
where ys = xs@W, yn = xn@W (b_shared cancels in training-mode BN),
ws = inv_s*gamma*u, wn = inv_n*gamma*v, inv = rsqrt(var+eps), and
C = sum_a[(beta - mean_s*inv_s*gamma)*u + (beta - mean_n*inv_n*gamma)*v] + b_out.

Device plan (SPMD over 8 cores, nodes sharded):
  phase 1: stream Xt (host-pretransposed, neigh in k-major [F,K,nodes] order,
           fp8e3 scaled x2 for halved HBM traffic; self stays fp16) ->
           yT = W^T Xt in PSUM (512-col subtiles, 2 accumulating matmuls each);
           ACT copies yT (with the 1/2 dequant folded into the copy scale) to a
           persistent fp16 SBUF store with sum(y) accum; DVE squares via
           tensor_tensor (2x mode) + tensor_scalar accumulate (4x mode).
  stats:   per-feature sums -> local mean/E[y^2] prescaled by 1/(8*rows);
           tiny AllReduce(add) across the 8 cores; wn/ws/C on-chip
           (rsqrt via exp(-0.5 ln); C reduced+broadcast by one ones-matmul).
  phase 2: per 128-node block: 32 matmuls (lhsT = CONTIGUOUS 128-col slices of
           the k-major yT store -> FWL fast weight load, rhs = wn) -> t PSUM
           [nodes, 32]; ACT relu(t + a_bias); exp; DVE row-sum + reciprocal;
           ACT scale; DMA out.
"""

import numpy as np
import ml_dtypes

import concourse.bass as bass
import concourse.mybir as mybir
import concourse.tile as tile
from concourse import bacc
from concourse.bass_utils import run_bass_kernel_spmd

N_CORES = 8
N_FULL, K, F, A = 20000, 32, 256, 128
BN_EPS = 1e-3
XSCALE = 2.0  # host premultiplies neigh feats before the e3m4 cast

F8 = mybir.dt.float8e3
F16 = mybir.dt.float16
F32 = mybir.dt.float32
AF = mybir.ActivationFunctionType

# Knobs for the test harness.
PROFILE = False
LAST_RESULT = None


def build_nc(nodes, k=K, f=F, a=A, n_cores=N_CORES, row_tile=4096, sub=512):
    """Build the per-core SPMD program. `nodes` = nodes per core."""
    assert f == 2 * 128 and a == 128
    rows_n = nodes * k
    rows_s = nodes
    nblk = (nodes + 127) // 128

    def tiles_of(rows):
        out = []
        r = 0
        while r < rows:
            out.append((r, min(row_tile, rows - r)))
            r += row_tile
        return out

    grp = 4 * sub  # 2048-col psum groups (4 banks)

    def pairs_of(rows):
        # (base, ns) for each up-to-2048-col psum group
        out = []
        for r0, nr in tiles_of(rows):
            s0 = 0
            while s0 < nr:
                ns = min(grp, nr - s0)
                out.append((r0 + s0, ns))
                s0 += ns
        return out

    np_n = len(pairs_of(rows_n))
    np_s = len(pairs_of(rows_s))

    nc = bacc.Bacc("TRN2", target_bir_lowering=False, num_devices=n_cores)
    xt_n = nc.declare_dram_parameter("xt_n", [f, rows_n], mybir.dt.uint8, isOutput=False)
    xt_s = nc.declare_dram_parameter("xt_s", [f, rows_s], F16, isOutput=False)
    w_lhsT = nc.declare_dram_parameter("w_lhsT", [2, 128, a], F16, isOutput=False)
    # params columns: gamma, v, u, b_out/A, beta*v, beta*u
    params = nc.declare_dram_parameter("params", [a, 6], F32, isOutput=False)
    out_d = nc.declare_dram_parameter("out", [rows_s, k], F32, isOutput=True)

    from contextlib import ExitStack

    with tile.TileContext(nc) as tc, ExitStack() as ctx:
        singles = ctx.enter_context(tc.tile_pool(name="singles", bufs=1))
        xn_pool = ctx.enter_context(tc.tile_pool(name="xn_pool", bufs=2))
        sq_pool = ctx.enter_context(tc.tile_pool(name="sq_pool", bufs=2))
        p2_pool = ctx.enter_context(tc.tile_pool(name="p2_pool", bufs=3))
        psum = ctx.enter_context(tc.tile_pool(name="psum", bufs=2, space="PSUM"))
        dram = ctx.enter_context(tc.tile_pool(name="dram", bufs=1, space="DRAM"))

        # ---- setup: params and weights
        w_sb = singles.tile([128, 2, a], F16)
        nc.sync.dma_start(out=w_sb, in_=w_lhsT.ap().rearrange("c p a -> p c a"))
        params_sb = singles.tile([a, 6], F32)
        nc.sync.dma_start(out=params_sb, in_=params.ap())
        eps_sb = singles.tile([a, 1], F32)
        nc.vector.memset(eps_sb, BN_EPS)
        ones_sb = singles.tile([a, 1], F32)
        nc.vector.memset(ones_sb, 1.0)
        ones_128 = singles.tile([a, 128], F16)
        nc.vector.memset(ones_128, 1.0)
        # warm the ACT function tables off the critical path; end with Ln so
        # its table set is resident when the post-collective chain issues Ln
        # (phase-1 Copy ops don't swap table sets)
        warm_sb = singles.tile([a, 1], F32)
        nc.scalar.activation(out=warm_sb, in_=ones_sb, func=AF.Exp)
        nc.scalar.activation(out=warm_sb, in_=ones_sb, func=AF.Relu)
        nc.scalar.activation(out=warm_sb, in_=ones_sb, func=AF.Ln)

        cv3 = singles.tile([a, 3], F32)
        nc.vector.tensor_copy(out=cv3[:, 2:3], in_=params_sb[:, 3:4])

        # ---- persistent stores (padded so 128-col lhsT slices never run out)
        yt_store = singles.tile([a, rows_n + 128], F16)
        ys_store = singles.tile([a, rows_s + 128], F16)

        # per-pair sum(y) columns (from the ACT copy's accum_out, already in
        # dequantized units and prescaled at reduce time) and per-pair
        # sum(y^2) columns (from the DVE square+accumulate pair)
        sum_n = singles.tile([a, np_n], F32)
        sum_s = singles.tile([a, np_s], F32)
        sq_n = singles.tile([a, np_n], F32)
        sq_s = singles.tile([a, np_s], F32)
        red_junk = singles.tile([a, max(np_n, np_s)], F32)

        # ---- phase 1: stream X^T, matmul into yT (paired 512-col subtiles per
        # 2-bank PSUM tile), one ACT copy (dequant scale + sum accum) per pair,
        # one DVE square-with-accum (scalar_tensor_tensor) per group
        def stream(xt_dram, dt, rows, store, sums, sqs, drain_scale, pool):
            view = xt_dram.ap().rearrange("(c p) r -> p c r", p=128)
            ipair = 0
            for r0, nr in tiles_of(rows):
                xt_t = pool.tile([128, 2, row_tile], dt, tag="xt")
                dst = xt_t[:, :, :nr]
                if dt == F8:
                    dst = dst.bitcast(mybir.dt.uint8)
                nc.sync.dma_start(out=dst, in_=view[:, :, r0 : r0 + nr])
                s0 = 0
                while s0 < nr:
                    ns = min(grp, nr - s0)
                    yt_psum = psum.tile([a, grp], F32, tag="g")
                    for q0 in range(0, ns, sub):
                        qn = min(sub, ns - q0)
                        for c in range(2):
                            nc.tensor.matmul(
                                yt_psum[:, q0 : q0 + qn],
                                w_sb[:, c, :],
                                xt_t[:, c, s0 + q0 : s0 + q0 + qn],
                                start=(c == 0),
                                stop=(c == 1),
                            )
                    base = r0 + s0
                    nc.scalar.activation(
                        out=store[:, base : base + ns],
                        in_=yt_psum[:, :ns],
                        func=AF.Copy,
                        scale=drain_scale,
                        accum_out=sums[:, ipair : ipair + 1],
                    )
                    src = store[:, base : base + ns]
                    scr = sq_pool.tile([a, grp], F16, tag="sq")
                    nc.vector.scalar_tensor_tensor(
                        out=scr[:, :ns],
                        in0=src,
                        scalar=1.0,
                        in1=src,
                        op0=mybir.AluOpType.mult,
                        op1=mybir.AluOpType.mult,
                        accum_out=sqs[:, ipair : ipair + 1],
                    )
                    ipair += 1
                    s0 += ns
            return ipair

        # allred_in layout: [mean_n, mean_s, E2_n, E2_s], prescaled by
        # 1/(n_cores*rows) so the AllReduce(add) yields global stats directly
        allred_in = singles.tile([a, 4], F32)

        def finish_stats(sums, sqs, n_pairs_used, rows, col):
            inv = 1.0 / (rows * n_cores)
            nc.vector.tensor_scalar(
                out=red_junk[:, :n_pairs_used],
                in0=sums[:, :n_pairs_used],
                scalar1=inv,
                scalar2=0.0,
                op0=mybir.AluOpType.mult,
                op1=mybir.AluOpType.add,
                accum_out=allred_in[:, col : col + 1],
            )
            nc.vector.tensor_scalar(
                out=red_junk[:, :n_pairs_used],
                in0=sqs[:, :n_pairs_used],
                scalar1=inv,
                scalar2=0.0,
                op0=mybir.AluOpType.mult,
                op1=mybir.AluOpType.add,
                accum_out=allred_in[:, col + 2 : col + 3],
            )

        # self stream first: its stats ops clear the DVE queue while the long
        # neigh stream runs, so the pre-collective DVE tail is minimal
        used_s = stream(xt_s, F16, rows_s, ys_store, sum_s, sq_s, 1.0, singles)
        finish_stats(sum_s, sq_s, used_s, rows_s, 1)
        used_n = stream(xt_n, F8, rows_n, yt_store, sum_n, sq_n, 1.0 / XSCALE, xn_pool)
        finish_stats(sum_n, sq_n, used_n, rows_n, 0)

        cc_in = dram.tile([a, 4], F32)
        cc_out = dram.tile([a, 4], F32)
        nc.sync.dma_start(out=cc_in, in_=allred_in)
        nc.gpsimd.collective_compute(
            "AllReduce",
            mybir.AluOpType.add,
            replica_groups=[list(range(n_cores))],
            ins=[cc_in.opt()],
            outs=[cc_out.opt()],
        )
        gs = singles.tile([a, 4], F32)
        nc.sync.dma_start(out=gs, in_=cc_out)

        # ---- global mean/var -> inv, w-vectors, constant C
        # params_sb columns: 0 gamma, 1 v, 2 u, 3 b_out/128, 4 beta*v, 5 beta*u
        # rsqrt via exp(-0.5*log(var+eps)) to stay in the Exp ACT table set
        # (avoids two Sqrt table-set switches on the critical path).
        gmean = gs[:, 0:2]
        # nvar = mean^2 - E2 (one STT per column); Ln(-1*nvar + eps) = ln(var+eps)
        nvar = singles.tile([a, 2], F32)
        for c in range(2):
            nc.vector.scalar_tensor_tensor(
                out=nvar[:, c : c + 1],
                in0=gmean[:, c : c + 1],
                scalar=gmean[:, c : c + 1],
                in1=gs[:, 2 + c : 3 + c],
                op0=mybir.AluOpType.mult,
                op1=mybir.AluOpType.subtract,
            )
        lv = singles.tile([a, 2], F32)
        nc.scalar.activation(out=lv, in_=nvar, func=AF.Ln, scale=-1.0, bias=eps_sb)
        inv = singles.tile([a, 2], F32)
        nc.scalar.activation(out=inv, in_=lv, func=AF.Exp, scale=-0.5)

        # wf = (inv * gamma) * [v | u] in one STT
        wf = singles.tile([a, 2], F32)  # col0: wn, col1: ws
        nc.vector.scalar_tensor_tensor(
            out=wf,
            in0=inv,
            scalar=params_sb[:, 0:1],
            in1=params_sb[:, 1:3],
            op0=mybir.AluOpType.mult,
            op1=mybir.AluOpType.mult,
        )
        w2_sb = singles.tile([a, 2], F16)
        nc.vector.tensor_copy(out=w2_sb, in_=wf)
        wn_sb = w2_sb[:, 0:1]
        ws_sb = w2_sb[:, 1:2]

        # C vector: beta*[v|u] - mean*wf  (mean*inv*gamma*[v|u] == gmean*wf)
        tmu = singles.tile([a, 2], F32)
        nc.vector.tensor_mul(tmu, gmean, wf)
        nc.vector.tensor_sub(cv3[:, 0:2], params_sb[:, 4:6], tmu)
        cvec = singles.tile([a, 1], F32)
        nc.vector.reduce_sum(out=cvec, in_=cv3, axis=mybir.AxisListType.X)
        cvec16 = singles.tile([a, 1], F16)
        nc.vector.tensor_copy(out=cvec16, in_=cvec)

        # reduce over partitions AND broadcast the scalar in one matmul:
        # out[m, 0] = sum_a ones[a, m] * cvec[a, 0]
        cb_tile = psum.tile([a, grp], F32, tag="g")
        nc.tensor.matmul(cb_tile[:, 0:1], ones_128, cvec16, start=True, stop=True)
        c_bcast = singles.tile([a, 1], F32)
        nc.vector.tensor_copy(out=c_bcast, in_=cb_tile[:, 0:1])

        # ---- phase 2: a_i = ys . ws + C, then t matmuls + softmax, per block
        # (the a-matmul is interleaved into the block loop so it pipelines
        # with the t-matmuls instead of forming a serial prologue).
        # lhsT slices are always full 128 contiguous cols (stores are padded)
        # so the compiler's FWL fast-weight-load stays enabled.
        a_all = singles.tile([128, nblk], F32)
        for b in range(nblk):
            b0 = b * 128
            nb = min(128, nodes - b0)
            blk_psum = psum.tile([128, grp], F32, tag="g")
            t_psum = blk_psum[:, 0:k]
            nc.tensor.matmul(
                blk_psum[:, sub : sub + 1],
                ys_store[:, b0 : b0 + 128],
                ws_sb,
                start=True,
                stop=True,
            )
            nc.vector.tensor_add(a_all[:, b : b + 1], blk_psum[:, sub : sub + 1], c_bcast)
            for kk in range(k):
                nc.tensor.matmul(
                    t_psum[:, kk : kk + 1],
                    yt_store[:, kk * nodes + b0 : kk * nodes + b0 + 128],
                    wn_sb,
                    start=True,
                    stop=True,
                )
            l_sb = p2_pool.tile([128, k], F32, tag="l")
            nc.scalar.activation(
                out=l_sb[:nb, :],
                in_=t_psum[:nb, :],
                func=AF.Relu,
                bias=a_all[:nb, b : b + 1],
            )
            e_sb = p2_pool.tile([128, k], F32, tag="e")
            nc.scalar.activation(out=e_sb[:nb, :], in_=l_sb[:nb, :], func=AF.Exp)
            ssum = p2_pool.tile([128, 1], F32, tag="ssum")
            nc.vector.reduce_sum(out=ssum[:nb, :], in_=e_sb[:nb, :], axis=mybir.AxisListType.X)
            rec = p2_pool.tile([128, 1], F32, tag="rec")
            nc.vector.reciprocal(out=rec[:nb, :], in_=ssum[:nb, :])
            coeff = p2_pool.tile([128, k], F32, tag="coeff")
            nc.scalar.activation(
                out=coeff[:nb, :], in_=e_sb[:nb, :], func=AF.Copy, scale=rec[:nb, :]
            )
            nc.sync.dma_start(out=out_d[b0 : b0 + nb, :], in_=coeff[:nb, :])

    nc.compile()
    return nc


_NC_CACHE = {}


def _get_nc(nodes, row_tile=4096):
    key = (nodes, row_tile)
    if key not in _NC_CACHE:
        _NC_CACHE[key] = build_nc(nodes, row_tile=row_tile)
    return _NC_CACHE[key]


def make_in_maps(self_feats, neigh_feats, W_shared, gamma, beta, W_out, b_out, n_cores=N_CORES):
    n = self_feats.shape[0]
    nodes = n // n_cores
    w_lhsT = np.stack([W_shared[:128], W_shared[128:]]).astype(np.float16)
    gamma = np.asarray(gamma, np.float32)
    beta = np.asarray(beta, np.float32)
    u = np.asarray(W_out[:A, 0], np.float32)
    v = np.asarray(W_out[A:, 0], np.float32)
    # columns: gamma, v, u, b_out/A, beta*v, beta*u
    params = np.stack(
        [
            gamma,
            v,
            u,
            np.full(A, np.float32(np.asarray(b_out).reshape(-1)[0]) / A),
            beta * v,
            beta * u,
        ],
        axis=1,
    ).astype(np.float32)
    in_maps = []
    for c in range(n_cores):
        sl = slice(c * nodes, (c + 1) * nodes)
        xs = np.asarray(self_feats[sl], np.float32)
        # k-major: [F, K, nodes] so phase-2 lhsT slices are contiguous
        xn = np.asarray(neigh_feats[sl], np.float32).transpose(2, 1, 0)
        xn8 = np.ascontiguousarray(xn.reshape(F, nodes * K) * XSCALE).astype(
            ml_dtypes.float8_e3m4
        )
        in_maps.append(
            {
                "xt_n": xn8.view(np.uint8),
                "xt_s": np.ascontiguousarray(xs.T).astype(np.float16),
                "w_lhsT": w_lhsT,
                "params": params,
            }
        )
    return in_maps


def kernel(self_feats, neigh_feats, W_shared, b_shared, gamma, beta, W_out, b_out):
    global LAST_RESULT
    self_feats = np.asarray(self_feats, np.float32)
    neigh_feats = np.asarray(neigh_feats, np.float32)
    W_shared = np.asarray(W_shared, np.float32)
    gamma = np.asarray(gamma, np.float32)
    beta = np.asarray(beta, np.float32)
    W_out = np.asarray(W_out, np.float32)
    b_out = np.asarray(b_out, np.float32)
    n = self_feats.shape[0]
    nodes = n // N_CORES
    nc = _get_nc(nodes)
    in_maps = make_in_maps(self_feats, neigh_feats, W_shared, gamma, beta, W_out, b_out)
    kw = {}
    if PROFILE:
        kw = dict(trace=True, trace_cores=[0])
    res = run_bass_kernel_spmd(nc, in_maps, list(range(N_CORES)), **kw)
    LAST_RESULT = res
    out = np.concatenate([res.results[c]["out"] for c in range(N_CORES)], axis=0)
    return out[:, :, None].astype(np.float32)
